# revision 19
# baseline (speedup 1.0000x reference)
"""Trainium2 Bass kernel for DynamicTaskMemoryInduction (capsule dynamic routing).

Math (reference semantics):
  Ws = W[0,:,0]  (W is a broadcast of shared weights over the in_caps axis C)
  hat_m[c,(n,d)] = m[c,:] @ Ws[(n,d),:]^T + b[0,n,c,d]      -> tm [C=64, N*D=768]
  hat_q[q,(n,d)] = q[q,:] @ Ws[(n,d),:]^T                   -> tq [Q, 768]  (c-independent)
  p = tanh(-pearson_d(tm, tq));  2x routing loop + final squash.

Because tq (and its routing updates) are c-independent, the per-(q,n,c,d)
tensors of the reference collapse to [Q,(n,d)] / [Q,(n,c)] shapes.

Key identities used on device (exact algebra, no approximation):
  - pearson numerator: num[q,n,c] = sum_d tm[n,c,d] * u[q,n,d] where
      u = tq - mean_d(tq) (centered), because sum_d u = 0.
  - recursive numerator: with u_i = lam_i * (tq_i - mean_d tq_i) (lam_i = 2^i),
      num'_{i+1} = num'_i + lam_i*(agree_i - mean_v_i * sm1)      (sm1 = sum_d tm)
      r_{i+1} = num' / sqrt(ssm * ssq(u) + lam^2 * EPS)
  - agree via the (constant) Gram matrix of tm:
      agree[q,n,c] = scale_v[q,n] * sum_{c'} coeff[q,n,c'] * G[n,c',c],
      G[n] = tm_n @ tm_n^T,  since v = scale_v * hat_v and hat_v = coeff @ tm_n.

All matmuls/transposes run as split-bf16 (x = hi + lo, both bf16; products
keep hi@hi + hi@lo + lo@hi, dropping lo@lo ~ 2^-18 relative): fp32/fp32r PE
matmuls execute ~100x slower than bf16 on this hardware path.

Sharding: data-parallel over Q across 8 cores (64 queries/core, q on SBUF
partitions). Ws/m/b replicated; hat_m recomputed on every core (it rides along
in the same matmul as hat_q: lhsT = [qT | mT] is exactly 128 columns).
"""

import numpy as np

EPS = 1e-8
Q, I, C, N, D = 512, 768, 64, 4, 192
ND, NC = N * D, N * C
NCORES = 8
QL = Q // NCORES  # 64 queries per core


def build(reps=1, stop_at="full"):
    import concourse.bacc as bacc
    import concourse.tile as tile
    import concourse.mybir as mybir
    import concourse.masks as masks

    F32 = mybir.dt.float32
    BF16 = mybir.dt.bfloat16
    AF = mybir.ActivationFunctionType
    OP = mybir.AluOpType
    AX = mybir.AxisListType

    nc = bacc.Bacc("TRN2", target_bir_lowering=False, debug=False,
                   num_devices=NCORES)

    wsh_d = nc.dram_tensor("ws_hi", [I, ND], BF16, kind="ExternalInput").ap()
    wsl_d = nc.dram_tensor("ws_lo", [I, ND], BF16, kind="ExternalInput").ap()
    qmh_d = nc.dram_tensor("qm_hi", [I, 128], BF16, kind="ExternalInput").ap()
    qml_d = nc.dram_tensor("qm_lo", [I, 128], BF16, kind="ExternalInput").ap()
    b_d = nc.dram_tensor("b_r", [C, ND], F32, kind="ExternalInput").ap()
    out_d = nc.dram_tensor("out", [QL, ND], F32, kind="ExternalOutput").ap()
    ssm_dr = nc.dram_tensor("ssm_dr", [C, N], F32).ap()
    s1m_dr = nc.dram_tensor("s1m_dr", [C, N], F32).ap()

    KC = I // 128  # 6 contraction chunks

    with tile.TileContext(nc) as tc:
        with tc.tile_pool(name="const", bufs=1) as cp, \
             tc.tile_pool(name="sb768", bufs=2) as sp768, \
             tc.tile_pool(name="sb256", bufs=2) as sp256, \
             tc.tile_pool(name="sbsm", bufs=2) as spsm, \
             tc.tile_pool(name="sbt", bufs=1) as spt:

            identb = cp.tile([128, 128], BF16, tag="identb")
            masks.make_identity(nc, identb[:])
            eps_t = {}
            for lam2 in (1.0, 4.0, 16.0):
                t = cp.tile([QL, 1], F32, tag=f"eps{lam2}")
                nc.gpsimd.memset(t[:], lam2 * EPS)
                eps_t[lam2] = t

            def split(x_ap, pool, tagbase, shape):
                """x (f32 AP) -> (hi, lo) bf16 tiles."""
                hi = pool.tile(shape, BF16, tag=f"{tagbase}h")
                nc.vector.tensor_copy(hi[:], x_ap)
                lo = pool.tile(shape, BF16, tag=f"{tagbase}l")
                nc.vector.tensor_sub(lo[:], x_ap, hi[:])
                return hi, lo

            for rep in range(reps):
                # ---------- load inputs ----------
                wsh, wsl, qmh, qml = [], [], [], []
                for k in range(KC):
                    sl = slice(k * 128, (k + 1) * 128)
                    for dsrc, lst, tg in ((wsh_d, wsh, "wsh"), (wsl_d, wsl, "wsl"),
                                          (qmh_d, qmh, "qmh"), (qml_d, qml, "qml")):
                        w_k = cp.tile([128, dsrc.shape[1]], BF16, tag=f"{tg}{k}")
                        nc.sync.dma_start(w_k[:], dsrc[sl, :])
                        lst.append(w_k)
                b_sb = cp.tile([C, ND], F32, tag="b")
                nc.sync.dma_start(b_sb[:], b_d[:])

                # ---------- phase A: [hat_q; hat_m] = qmT.T @ wsT (split) ----
                with tc.tile_pool(name="psA", bufs=1, space="PSUM") as psA:
                    ps_a = psA.tile([128, ND], F32, tag="a")
                    terms = [(qmh, wsh), (qmh, wsl), (qml, wsh)]
                    nmm = KC * len(terms)
                    for c0, c1 in ((0, 512), (512, 768)):
                        i_mm = 0
                        for k in range(KC):
                            for lh, rh in terms:
                                nc.tensor.matmul(ps_a[:, c0:c1], lh[k][:],
                                                 rh[k][:, c0:c1],
                                                 start=(i_mm == 0),
                                                 stop=(i_mm == nmm - 1))
                                i_mm += 1

                    # tm = hat_m + b ; u0 = centered hat_q
                    tm = cp.tile([C, ND], F32, tag="tm")
                    nc.vector.tensor_add(tm[:], ps_a[64:128, :], b_sb[:])

                    s1q = spsm.tile([QL, N], F32, tag="s1q")
                    nc.vector.tensor_reduce(
                        out=s1q[:], in_=ps_a[0:64, :].rearrange("p (n d) -> p n d", n=N),
                        axis=AX.X, op=OP.add)
                    muq = spsm.tile([QL, N], F32, tag="muq")
                    nc.vector.tensor_scalar_mul(muq[:], s1q[:], 1.0 / D)
                    u = sp768.tile([QL, ND], F32, tag="u")
                    nc.vector.tensor_sub(
                        u[:].rearrange("p (n d) -> p n d", n=N),
                        ps_a[0:64, :].rearrange("p (n d) -> p n d", n=N),
                        muq[:].unsqueeze(2).broadcast_to([QL, N, D]))

                if stop_at == "phaseA":
                    nc.sync.dma_start(out_d[:], tm[:])
                    continue

                tm_h, tm_l = split(tm[:], cp, "tms", [C, ND])
                u_h, u_l = split(u[:], spt, "us", [QL, ND])

                # ssq0 = sum_d u^2 per n
                squ = sp768.tile([QL, ND], F32, tag="squ")
                nc.vector.tensor_mul(squ[:], u[:], u[:])
                ssq = spsm.tile([QL, N], F32, tag="ssq")
                nc.vector.tensor_reduce(
                    out=ssq[:], in_=squ[:].rearrange("p (n d) -> p n d", n=N),
                    axis=AX.X, op=OP.add)

                # ---------- tm statistics ----------
                s1m = spsm.tile([C, N], F32, tag="s1m")
                nc.vector.tensor_reduce(
                    out=s1m[:], in_=tm[:].rearrange("p (n d) -> p n d", n=N),
                    axis=AX.X, op=OP.add)
                mum = spsm.tile([C, N], F32, tag="mum")
                nc.vector.tensor_scalar_mul(mum[:], s1m[:], 1.0 / D)
                tmc = sp768.tile([C, ND], F32, tag="tmc")
                nc.vector.tensor_sub(
                    tmc[:].rearrange("p (n d) -> p n d", n=N),
                    tm[:].rearrange("p (n d) -> p n d", n=N),
                    mum[:].unsqueeze(2).broadcast_to([C, N, D]))
                sqm = sp768.tile([C, ND], F32, tag="squ")
                nc.vector.tensor_mul(sqm[:], tmc[:], tmc[:])
                ssm = spsm.tile([C, N], F32, tag="ssm")
                nc.vector.tensor_reduce(
                    out=ssm[:], in_=sqm[:].rearrange("p (n d) -> p n d", n=N),
                    axis=AX.X, op=OP.add)

                # ssm,s1m [64(c),4(n)] -> bounce through DRAM with a transposing
                # read -> [1,(n,c)] row -> gpsimd partition_broadcast (exact fp32).
                ssm_b = cp.tile([QL, NC], F32, tag="ssm_b")
                sm1_b = cp.tile([QL, NC], F32, tag="sm1_b")
                for src, dst, drt in ((ssm, ssm_b, ssm_dr), (s1m, sm1_b, s1m_dr)):
                    nc.sync.dma_start(out=drt[:], in_=src[:])
                    row = spsm.tile([1, NC], F32, tag="row")
                    nc.sync.dma_start(
                        out=row[:].rearrange("x (n c) -> x n c", n=N),
                        in_=drt[:].rearrange("c n -> n c"))
                    nc.gpsimd.partition_broadcast(dst[:], row[:])

                with tc.tile_pool(name="psT", bufs=2, space="PSUM") as psT, \
                     tc.tile_pool(name="psB", bufs=2, space="PSUM") as psB:
                    # transposed tiles per d-chunk: A = d 0:128, B = d 128:192,
                    # for hi and lo; columns packed [d, (n,*)] with n at cols n*64.
                    def tr_blocks(hi, lo, pool, pfx):
                        res = {}
                        for cname, off, w in (("A", 0, 128), ("B", 128, 64)):
                            for sname, src in (("h", hi), ("l", lo)):
                                pt = psT.tile([128, NC], BF16, tag=f"tr{cname}")
                                for n in range(N):
                                    nc.tensor.transpose(
                                        pt[:w, n * C:(n + 1) * C],
                                        src[:, n * D + off:n * D + off + w],
                                        identb[:64, :64])
                                t_b = pool.tile([w, NC], BF16,
                                                tag=f"{pfx}{cname}{sname}")
                                nc.vector.tensor_copy(t_b[:], pt[:w, :])
                                res[cname + sname] = t_b
                        return res

                    tmT = tr_blocks(tm_h, tm_l, cp, "tmT")
                    uT = tr_blocks(u_h, u_l, spt, "uT")

                    def mm3_blocks(out_ps, Lt, Rt, n):
                        """accumulate split product over d-chunks A,B for block n"""
                        sl = (slice(None), slice(n * C, (n + 1) * C))
                        combos = [("A", "h", "h"), ("A", "h", "l"), ("A", "l", "h"),
                                  ("B", "h", "h"), ("B", "h", "l"), ("B", "l", "h")]
                        for j, (cn, a, bside) in enumerate(combos):
                            w = 128 if cn == "A" else 64
                            nc.tensor.matmul(out_ps[sl],
                                             Lt[cn + a][:w, n * C:(n + 1) * C],
                                             Rt[cn + bside][:w, n * C:(n + 1) * C],
                                             start=(j == 0), stop=(j == len(combos) - 1))

                    # gram G[n] = tm_n @ tm_n^T
                    pg = psB.tile([C, NC], F32, tag="blk")
                    for n in range(N):
                        mm3_blocks(pg, tmT, tmT, n)
                    g_h, g_l = split(pg[:], cp, "gs", [C, NC])

                    # pear #1: num0[q,(n,c)] = sum_d u0T[d,q] * tmT[d,c]
                    pp = psB.tile([QL, NC], F32, tag="blk")
                    for n in range(N):
                        mm3_blocks(pp, uT, tmT, n)
                    num = sp256.tile([QL, NC], F32, tag="num")
                    nc.vector.tensor_copy(num[:], pp[:])

                if stop_at == "setup":
                    nc.sync.dma_start(out_d[:], u[:])
                    continue

                def make_p(num_t, ssq_t, lam):
                    """p = tanh(-num / sqrt(ssm*ssq + lam^2*EPS)) ; [64,256]"""
                    den2 = sp256.tile([QL, NC], F32, tag="den2")
                    nc.vector.tensor_mul(
                        den2[:].rearrange("p (n c) -> p n c", n=N),
                        ssm_b[:].rearrange("p (n c) -> p n c", n=N),
                        ssq_t[:].unsqueeze(2).broadcast_to([QL, N, C]))
                    den = sp256.tile([QL, NC], F32, tag="den")
                    nc.scalar.activation(den[:], den2[:], AF.Sqrt,
                                         bias=eps_t[lam * lam][:], scale=1.0)
                    inv = sp256.tile([QL, NC], F32, tag="inv")
                    nc.vector.reciprocal(inv[:], den[:])
                    r_t = sp256.tile([QL, NC], F32, tag="r")
                    nc.vector.tensor_mul(r_t[:], num_t[:], inv[:])
                    p_t = sp256.tile([QL, NC], F32, tag="p")
                    nc.scalar.activation(p_t[:], r_t[:], AF.Tanh, bias=0.0, scale=-1.0)
                    return p_t

                def softmax_n(a_t):
                    """softmax over n of a [64,(n,c)] -> d_sm [64,256]"""
                    amax = spsm.tile([QL, C], F32, tag="amax")
                    nc.vector.tensor_reduce(
                        out=amax[:], in_=a_t[:].rearrange("p (n c) -> p c n", n=N),
                        axis=AX.X, op=OP.max)
                    e_t = sp256.tile([QL, NC], F32, tag="e")
                    nc.vector.tensor_sub(
                        e_t[:].rearrange("p (n c) -> p n c", n=N),
                        a_t[:].rearrange("p (n c) -> p n c", n=N),
                        amax[:].unsqueeze(1).broadcast_to([QL, N, C]))
                    nc.scalar.activation(e_t[:], e_t[:], AF.Exp, bias=0.0, scale=1.0)
                    rs = spsm.tile([QL, C], F32, tag="rs")
                    nc.vector.tensor_reduce(
                        out=rs[:], in_=e_t[:].rearrange("p (n c) -> p c n", n=N),
                        axis=AX.X, op=OP.add)
                    rsi = spsm.tile([QL, C], F32, tag="rsi")
                    nc.vector.reciprocal(rsi[:], rs[:])
                    d_sm = sp256.tile([QL, NC], F32, tag="dsm")
                    nc.vector.tensor_mul(
                        d_sm[:].rearrange("p (n c) -> p n c", n=N),
                        e_t[:].rearrange("p (n c) -> p n c", n=N),
                        rsi[:].unsqueeze(1).broadcast_to([QL, N, C]))
                    return d_sm

                p_t = make_p(num, ssq, 1.0)
                a_t = None

                with tc.tile_pool(name="psI", bufs=2, space="PSUM") as psI, \
                     tc.tile_pool(name="psH", bufs=2, space="PSUM") as psH:

                    def coeff_mm(coeff_t):
                        """split coeff, transpose blocks, split-matmul hv."""
                        c_h, c_l = split(coeff_t[:], sp256, "cs", [QL, NC])
                        cT = {}
                        for sname, src in (("h", c_h), ("l", c_l)):
                            pc = psI.tile([64, NC], BF16, tag="ctr")
                            for n in range(N):
                                nc.tensor.transpose(pc[:, n * C:(n + 1) * C],
                                                    src[:, n * C:(n + 1) * C],
                                                    identb[:64, :64])
                            t_c = sp256.tile([64, NC], BF16, tag=f"cT{sname}")
                            nc.vector.tensor_copy(t_c[:], pc[:])
                            cT[sname] = t_c
                        hv01 = psH.tile([QL, 2 * D], F32, tag="hv01")
                        hv23 = psH.tile([QL, 2 * D], F32, tag="hv23")
                        hv = [(hv01, 0), (hv01, 1), (hv23, 0), (hv23, 1)]
                        for n in range(N):
                            t, half = hv[n]
                            osl = (slice(None), slice(half * D, (half + 1) * D))
                            csl = (slice(None), slice(n * C, (n + 1) * C))
                            dsl = (slice(None), slice(n * D, (n + 1) * D))
                            nc.tensor.matmul(t[osl], cT["h"][csl], tm_h[dsl],
                                             start=True, stop=False)
                            nc.tensor.matmul(t[osl], cT["h"][csl], tm_l[dsl],
                                             start=False, stop=False)
                            nc.tensor.matmul(t[osl], cT["l"][csl], tm_h[dsl],
                                             start=False, stop=True)
                        return cT, hv

                    def agree_mm(cT):
                        pag = psI.tile([QL, NC], F32, tag="ag")
                        for n in range(N):
                            csl = (slice(None), slice(n * C, (n + 1) * C))
                            nc.tensor.matmul(pag[csl], cT["h"][csl], g_h[csl],
                                             start=True, stop=False)
                            nc.tensor.matmul(pag[csl], cT["h"][csl], g_l[csl],
                                             start=False, stop=False)
                            nc.tensor.matmul(pag[csl], cT["l"][csl], g_h[csl],
                                             start=False, stop=True)
                        return pag

                    def squash_stats(hv):
                        """returns scale_v [64,4], s1hv [64,4]"""
                        s1hv = spsm.tile([QL, N], F32, tag="s1hv")
                        sshv = spsm.tile([QL, N], F32, tag="sshv")
                        for n in range(N):
                            t, half = hv[n]
                            sl = t[:, half * D:(half + 1) * D]
                            nc.vector.tensor_reduce(out=s1hv[:, n:n + 1], in_=sl,
                                                    axis=AX.X, op=OP.add)
                            junk = sp768.tile([QL, D], F32, tag="junk")
                            nc.scalar.activation(junk[:], sl, AF.Square,
                                                 accum_out=sshv[:, n:n + 1])
                        t1 = spsm.tile([QL, N], F32, tag="t1")
                        nc.vector.tensor_scalar_add(t1[:], sshv[:], 1.0)
                        t1r = spsm.tile([QL, N], F32, tag="t1r")
                        nc.vector.reciprocal(t1r[:], t1[:])
                        t2 = spsm.tile([QL, N], F32, tag="t2")
                        nc.vector.tensor_mul(t2[:], sshv[:], t1r[:])
                        ds = spsm.tile([QL, N], F32, tag="ds")
                        nc.scalar.activation(ds[:], sshv[:], AF.Sqrt,
                                             bias=eps_t[1.0][:], scale=1.0)
                        dsr = spsm.tile([QL, N], F32, tag="dsr")
                        nc.vector.reciprocal(dsr[:], ds[:])
                        scale = spsm.tile([QL, N], F32, tag="scale")
                        nc.vector.tensor_mul(scale[:], t2[:], dsr[:])
                        return scale, s1hv

                    lam = 1.0
                    for it in (1, 2):
                        coeff = sp256.tile([QL, NC], F32, tag="coeff")
                        if it == 1:
                            nc.vector.tensor_scalar_add(coeff[:], p_t[:], 1.0 / N)
                        else:
                            d_sm = softmax_n(a_t)
                            nc.vector.tensor_add(coeff[:], d_sm[:], p_t[:])

                        cT, hv = coeff_mm(coeff)
                        pag = agree_mm(cT)
                        scale, s1hv = squash_stats(hv)

                        # agree = scale_v (bcast c) * pag
                        agree = sp256.tile([QL, NC], F32, tag="agree")
                        nc.vector.tensor_mul(
                            agree[:].rearrange("p (n c) -> p n c", n=N),
                            pag[:].rearrange("p (n c) -> p n c", n=N),
                            scale[:].unsqueeze(2).broadcast_to([QL, N, C]))

                        # a update: a += p * agree
                        pa = sp256.tile([QL, NC], F32, tag="pa")
                        nc.vector.tensor_mul(pa[:], p_t[:], agree[:])
                        if it == 1:
                            a_t = pa
                        else:
                            a_new = sp256.tile([QL, NC], F32, tag="a")
                            nc.vector.tensor_add(a_new[:], a_t[:], pa[:])
                            a_t = a_new

                        # mean_v = scale * s1hv / D
                        mv = spsm.tile([QL, N], F32, tag="mv")
                        nc.vector.tensor_mul(mv[:], scale[:], s1hv[:])
                        nc.vector.tensor_scalar_mul(mv[:], mv[:], 1.0 / D)

                        # num' += lam * (agree - mv*sm1)
                        q1 = sp256.tile([QL, NC], F32, tag="q1")
                        nc.vector.tensor_mul(
                            q1[:].rearrange("p (n c) -> p n c", n=N),
                            sm1_b[:].rearrange("p (n c) -> p n c", n=N),
                            mv[:].unsqueeze(2).broadcast_to([QL, N, C]))
                        q2 = sp256.tile([QL, NC], F32, tag="q2")
                        nc.vector.tensor_sub(q2[:], agree[:], q1[:])
                        num_new = sp256.tile([QL, NC], F32, tag="num")
                        nc.vector.scalar_tensor_tensor(
                            out=num_new[:], in0=q2[:], scalar=lam, in1=num[:],
                            op0=OP.mult, op1=OP.add)
                        num = num_new

                        # w1 = v - mv = hv*scale - mv (per n) ; u += lam*w1
                        w1 = sp768.tile([QL, ND], F32, tag="w1")
                        for n in range(N):
                            t, half = hv[n]
                            nc.vector.tensor_scalar(
                                out=w1[:, n * D:(n + 1) * D],
                                in0=t[:, half * D:(half + 1) * D],
                                scalar1=scale[:, n:n + 1], scalar2=mv[:, n:n + 1],
                                op0=OP.mult, op1=OP.subtract)
                        u_new = sp768.tile([QL, ND], F32, tag="u")
                        nc.vector.scalar_tensor_tensor(
                            out=u_new[:], in0=w1[:], scalar=lam, in1=u[:],
                            op0=OP.mult, op1=OP.add)
                        u = u_new
                        lam *= 2.0

                        squ2 = sp768.tile([QL, ND], F32, tag="squ")
                        nc.vector.tensor_mul(squ2[:], u[:], u[:])
                        ssq2 = spsm.tile([QL, N], F32, tag="ssq")
                        nc.vector.tensor_reduce(
                            out=ssq2[:], in_=squ2[:].rearrange("p (n d) -> p n d", n=N),
                            axis=AX.X, op=OP.add)
                        p_t = make_p(num, ssq2, lam)

                    # ---------- final: d=softmax(a), hv3, squash -> out ----------
                    d_sm = softmax_n(a_t)
                    coeff = sp256.tile([QL, NC], F32, tag="coeff")
                    nc.vector.tensor_add(coeff[:], d_sm[:], p_t[:])
                    cT, hv = coeff_mm(coeff)
                    scale, _s1 = squash_stats(hv)
                    out_sb = sp768.tile([QL, ND], F32, tag="out")
                    for n in range(N):
                        t, half = hv[n]
                        nc.vector.tensor_scalar_mul(
                            out_sb[:, n * D:(n + 1) * D],
                            t[:, half * D:(half + 1) * D], scale[:, n:n + 1])
                    nc.sync.dma_start(out_d[:], out_sb[:])

    nc.compile()
    return nc


_BUILD_CACHE = {}


def _get_built(reps=1):
    if reps not in _BUILD_CACHE:
        _BUILD_CACHE[reps] = build(reps)
    return _BUILD_CACHE[reps]


def _split_np(x):
    import ml_dtypes
    hi = x.astype(ml_dtypes.bfloat16)
    lo = (x - hi.astype(np.float32)).astype(ml_dtypes.bfloat16)
    return hi, lo


def _prep_inputs(m, q, W, b):
    """Host-side layout prep + per-core sharding."""
    m = np.asarray(m, dtype=np.float32)
    q = np.asarray(q, dtype=np.float32)
    W = np.asarray(W, dtype=np.float32)
    b = np.asarray(b, dtype=np.float32)
    Ws = W[0, :, 0, :, :].reshape(ND, I)          # [N*D, I]
    wsT = np.ascontiguousarray(Ws.T)              # [I, N*D]
    ws_hi, ws_lo = _split_np(wsT)
    mT = m.T                                      # [I, C]
    b_r = np.ascontiguousarray(b[0].transpose(1, 0, 2).reshape(C, ND))
    in_maps = []
    for c in range(NCORES):
        qc = q[c * QL:(c + 1) * QL, :]            # [QL, I]
        qmT = np.ascontiguousarray(np.concatenate([qc.T, mT], axis=1))  # [I, 128]
        qm_hi, qm_lo = _split_np(qmT)
        in_maps.append({"ws_hi": ws_hi, "ws_lo": ws_lo,
                        "qm_hi": qm_hi, "qm_lo": qm_lo, "b_r": b_r})
    return in_maps


def kernel(m, q, W, b):
    from concourse.bass_utils import run_bass_kernel_spmd
    nc = _get_built(1)
    in_maps = _prep_inputs(m, q, W, b)
    res = run_bass_kernel_spmd(nc, in_maps, list(range(NCORES)))
    out = np.concatenate([res.results[c]["out"] for c in range(NCORES)], axis=0)
    return out.astype(np.float32)
